# revision 5
# baseline (speedup 1.0000x reference)
"""Chamfer distance kernel for Trainium2, 8 NeuronCores.

Problem: B=4 batches, N=M=8192 points, C=3 coords.
  dist1[b,n] = min_m ||xyz1[b,n]-xyz2[b,m]||^2 ;  dist2[b,m] = min_n ||...||^2

Sharding: 4 batches x 2 directions = 8 perfectly balanced SPMD core tasks.
Each core solves one (query=A[8192], database=B[8192]) brute-force pass.

Per-core algorithm (v2):
  min_m d[n,m] = sq1[n] + min_m( sq2[m] - 2*x[n].y[m] )
The inner expression is computed on the TensorEngine as a K=15 bf16 matmul
(hi/lo bf16 split of coordinates => ~fp32 accuracy at full PE speed) into
1024-column PSUM megatiles (2 banks each, 3 rotating slots).

The row-min runs as a DVE/ScalarE pipeline at 2 elems/cycle/lane on the
DVE (2x the tensor_reduce path the v1 kernel used):
  - ScalarE drains even megatiles PSUM->SBUF (1 elem/cyc @ 1.2 GHz)
  - DVE TENSOR_TENSOR_SCAN (min/min) pairs each odd PSUM megatile with
    the preceding SBUF copy: state = min(psum[t], state, sbuf[t]),
    chained across megatiles via `initial`, with a stride-0 broadcast
    out so the final state lands directly in the accumulator column.
    One scan consumes 2048 elems/lane in ~1024 cycles.
Every d element is drained from PSUM exactly once (half by ScalarE, half
by the DVE), which is the engine-bandwidth floor for this problem.

Features are built wide ([128, 64*16] bf16, 16 slots per point), then
PE-transposed in [128,16] -> [16,128] chunks into [16, 1024] PSUM staging
tiles and copied by ScalarE into stA/stB [16, 8192] (partition base 0 so
lhsT and rhs share base).  sq1 stays wide [128,64] and is added at the
end; the output DMA un-permutes the (p,k) query order.
"""

import sys
import copy
import contextlib

sys.path.insert(0, "/opt/trn_rl_repo")

import numpy as np

import bass_rust
import concourse.bass as bass
import concourse.tile as tile
from concourse import mybir
from concourse.bass_utils import run_bass_kernel_spmd

F32 = mybir.dt.float32
BF16 = mybir.dt.bfloat16

N = 8192          # query points per core
M = 8192          # database points per core
P = 128           # partitions
Q = N // P        # 64 points per partition stripe (n = p*Q + k)
S = 16            # feature slots per point (15 used)
K = 15            # matmul contraction rows actually used
CHUNK = 512       # matmul moving free dim (one PSUM bank of fp32)
MT = 1024         # megatile free dim (2 PSUM banks)
NMT = M // MT     # 8 megatiles per row-tile
NROWS = Q         # row-tiles to process (tunable for benchmarking only)
BIG = 3.0e38      # min-accumulator init


def _split_excess_waits(nc, max_waits=1):
    # This container's walrus codegen only supports a single sem-wait
    # command per instruction ("Too many sync wait commands"). Hoist excess
    # sem waits onto NoOps inserted just before the offender on the same
    # engine (program order preserves blocking semantics).
    n_split = 0
    for f in nc.m.functions:
        for b in f.blocks:
            insts = b.instructions
            for ins in list(insts):
                si = ins.sync_info
                if si is None:
                    continue
                w = list(si.on_wait)
                if len(w) <= max_waits:
                    continue
                idx = insts.index(ins)
                keep = w[-max_waits:]
                extra = w[:-max_waits]
                ins.sync_info = bass_rust.SyncInfo(
                    on_wait=keep, on_update=list(si.on_update)
                )
                for j, wt in enumerate(extra):
                    c = bass_rust.InstNoOp(name=f"{ins.name}-wsplit{j}", ins=[], outs=[])
                    c.engine = ins.engine
                    c.sync_info = bass_rust.SyncInfo(on_wait=[wt], on_update=[])
                    insts.insert(idx + j, c)
                    n_split += 1
    return n_split


def _prep_side(nc, pool, xyz_dram, side):
    """DMA [8192,3] f32 -> wide layout, build bf16 hi/lo feature slots,
    return (feat_wide [128, Q*S] bf16, sq_wide [128, Q] f32 or None).

    Feature slot layout per point (slot index f in [0,S)):
      A side (lhsT rows): 0-2: ones,  3-5: xh,  6-8: xh,  9-11: xl, 12-14: xl
      B side (rhs rows):  0-2: sq h/m/l,  3-5: -2yh, 6-8: -2yl,
                          9-11: -2yh, 12-14: -2yl
    """
    v = nc.vector

    w = pool.tile([P, Q * 3], F32, tag=f"{side}_w")
    nc.sync.dma_start(w[:], xyz_dram.rearrange("(p k) c -> p (k c)", p=P))

    feat = pool.tile([P, Q * S], BF16, tag=f"{side}_feat")
    v.memset(feat[:], 0.0)
    f3 = feat[:].rearrange("p (k s) -> p k s", s=S)

    hi_b = pool.tile([P, Q * 3], BF16, tag=f"{side}_hib")
    v.tensor_copy(hi_b[:], w[:])                       # round to bf16
    hi_f = pool.tile([P, Q * 3], F32, tag=f"{side}_hif")
    v.tensor_copy(hi_f[:], hi_b[:])                    # exact back to f32
    lo_f = pool.tile([P, Q * 3], F32, tag=f"{side}_lof")
    v.tensor_tensor(lo_f[:], w[:], hi_f[:], op=mybir.AluOpType.subtract)
    lo_b = pool.tile([P, Q * 3], BF16, tag=f"{side}_lob")
    v.tensor_copy(lo_b[:], lo_f[:])                    # round residual to bf16
    lo_xf = pool.tile([P, Q * 3], F32, tag=f"{side}_loxf")
    v.tensor_copy(lo_xf[:], lo_b[:])                   # exact f32 of bf16 lo

    # x_hat = hi + lo  (exact in f32; <=18 mantissa bits)
    hat = pool.tile([P, Q * 3], F32, tag=f"{side}_hat")
    v.tensor_tensor(hat[:], hi_f[:], lo_xf[:], op=mybir.AluOpType.add)
    # sq = sum_c x_hat_c^2
    prod = pool.tile([P, Q * 3], F32, tag=f"{side}_prod")
    v.tensor_tensor(prod[:], hat[:], hat[:], op=mybir.AluOpType.mult)
    sq = pool.tile([P, Q], F32, tag=f"{side}_sq")
    v.tensor_reduce(
        sq[:],
        prod[:].rearrange("p (k c) -> p k c", c=3),
        axis=mybir.AxisListType.X,
        op=mybir.AluOpType.add,
    )

    if side == "a":
        v.memset(f3[:, :, 0:3], 1.0)
        v.tensor_copy(f3[:, :, 3:6], hi_b[:].rearrange("p (k c) -> p k c", c=3))
        v.tensor_copy(f3[:, :, 6:9], hi_b[:].rearrange("p (k c) -> p k c", c=3))
        v.tensor_copy(f3[:, :, 9:12], lo_b[:].rearrange("p (k c) -> p k c", c=3))
        v.tensor_copy(f3[:, :, 12:15], lo_b[:].rearrange("p (k c) -> p k c", c=3))
        return feat, sq
    else:
        # -2*hi and -2*lo, exact scalings of bf16 values
        hi3 = hi_f[:].rearrange("p (k c) -> p k c", c=3)
        lo3 = lo_xf[:].rearrange("p (k c) -> p k c", c=3)
        v.tensor_scalar_mul(f3[:, :, 3:6], hi3, -2.0)
        v.tensor_scalar_mul(f3[:, :, 9:12], hi3, -2.0)
        v.tensor_scalar_mul(f3[:, :, 6:9], lo3, -2.0)
        v.tensor_scalar_mul(f3[:, :, 12:15], lo3, -2.0)
        # 3-way bf16 split of sq2 into slots 0..2
        v.tensor_copy(f3[:, :, 0:1], sq[:].rearrange("p (k o) -> p k o", o=1))
        s_hf = pool.tile([P, Q], F32, tag="b_shf")
        v.tensor_copy(s_hf[:], f3[:, :, 0:1])
        r1 = pool.tile([P, Q], F32, tag="b_r1")
        v.tensor_tensor(r1[:], sq[:], s_hf[:], op=mybir.AluOpType.subtract)
        v.tensor_copy(f3[:, :, 1:2], r1[:].rearrange("p (k o) -> p k o", o=1))
        s_mf = pool.tile([P, Q], F32, tag="b_smf")
        v.tensor_copy(s_mf[:], f3[:, :, 1:2])
        r2 = pool.tile([P, Q], F32, tag="b_r2")
        v.tensor_tensor(r2[:], r1[:], s_mf[:], op=mybir.AluOpType.subtract)
        v.tensor_copy(f3[:, :, 2:3], r2[:].rearrange("p (k o) -> p k o", o=1))
        return feat, None


def build_nc(repeat=1):
    nc = bass.Bass()
    a_xyz = nc.dram_tensor("a_xyz", [N, 3], F32, kind="ExternalInput")
    b_xyz = nc.dram_tensor("b_xyz", [M, 3], F32, kind="ExternalInput")
    ident = nc.dram_tensor("ident", [P, P], BF16, kind="ExternalInput")
    out = nc.dram_tensor("dist", [N], F32, kind="ExternalOutput")

    with tile.TileContext(nc) as tc:
        with contextlib.ExitStack() as stack:
            if repeat > 1:
                stack.enter_context(tc.For_i(0, repeat, 1))
            prep = stack.enter_context(tc.tile_pool(name="prep", bufs=1))
            stage = stack.enter_context(tc.tile_pool(name="stage", bufs=1))
            res = stack.enter_context(tc.tile_pool(name="res", bufs=1))
            cs = stack.enter_context(tc.tile_pool(name="cs", bufs=4))
            feat_b, _ = _prep_side(nc, prep, b_xyz, "b")
            feat_a, sq1 = _prep_side(nc, prep, a_xyz, "a")

            id_t = stage.tile([P, P], BF16, tag="ident")
            nc.sync.dma_start(id_t[:], ident[:])
            stA = stage.tile([S, Q * P], BF16, tag="stA")   # [16, 8192]
            stB = stage.tile([S, Q * P], BF16, tag="stB")

            # Transpose wide features into matmul layout: PE transposes
            # [128,16]->[16,128] into [16,1024] PSUM staging (1 bank),
            # ScalarE copies each staged group of 8 points into stA/stB.
            with tc.tile_pool(name="tstg", bufs=2, space="PSUM") as tstg:

                def stage_group(feat, st, h):
                    stg = tstg.tile([S, 8 * P], BF16, tag="stg", name="stg")
                    for t in range(8):
                        k = 8 * h + t
                        nc.tensor.transpose(
                            stg[:, t * P : (t + 1) * P],
                            feat[:, k * S : (k + 1) * S],
                            id_t[:],
                        )
                    nc.scalar.copy(st[:, h * 8 * P : (h + 1) * 8 * P], stg[:])

                for h in range(Q // 8):
                    stage_group(feat_b, stB, h)
                for h in range(Q // 8):
                    stage_group(feat_a, stA, h)

            pp = stack.enter_context(tc.tile_pool(name="psum", bufs=4, space="PSUM"))

            # Per-row-tile min accumulator: acc[:, r] accumulates the row-min
            # via chained min-scans (broadcast out lands the final scan state
            # directly in the accumulator slot).
            acc = res.tile([P, Q], F32, tag="acc")

            # Main loop over row-tiles (query groups)
            for r in range(NROWS):
                lhsT = stA[0:K, r * P : (r + 1) * P]
                pend = None
                first = True
                for g in range(NMT):
                    ps = pp.tile([P, MT], F32, tag="ps")
                    for c in range(MT // CHUNK):
                        cc = g * MT + c * CHUNK
                        nc.tensor.matmul(
                            ps[:, c * CHUNK : (c + 1) * CHUNK],
                            lhsT, stB[0:K, cc : cc + CHUNK],
                            start=True, stop=True,
                        )
                    if g % 2 == 0:
                        s = cs.tile([P, MT], F32, tag="cp")
                        nc.scalar.copy(s[:], ps[:])
                        pend = s
                    else:
                        nc.vector.tensor_tensor_scan(
                            acc[:, r : r + 1].broadcast_to((P, MT)),
                            ps[:], pend[:],
                            initial=(BIG if first else acc[:, r : r + 1]),
                            op0=mybir.AluOpType.min, op1=mybir.AluOpType.min,
                        )
                        first = False

            # add sq1, write out (un-permute the (p,k) query order)
            dist = res.tile([P, Q], F32, tag="dist")
            nc.vector.tensor_tensor(
                dist[:], acc[:], sq1[:], op=mybir.AluOpType.add
            )
            nc.sync.dma_start(out.rearrange("(p k) -> p k", p=P), dist[:])

    _split_excess_waits(nc)
    return nc


_NC_CACHE = {}


def _get_nc(repeat=1):
    if repeat not in _NC_CACHE:
        _NC_CACHE[repeat] = build_nc(repeat)
    return _NC_CACHE[repeat]


def kernel(xyz1, xyz2, _trace=False, _repeat=1):
    xyz1 = np.ascontiguousarray(np.asarray(xyz1, dtype=np.float32))
    xyz2 = np.ascontiguousarray(np.asarray(xyz2, dtype=np.float32))
    B = xyz1.shape[0]
    assert xyz1.shape == (B, N, 3) and xyz2.shape == (B, M, 3)

    nc = _get_nc(_repeat)
    import ml_dtypes
    ident = np.eye(P, dtype=ml_dtypes.bfloat16)
    in_maps = []
    for c in range(2 * B):
        b, d = c % B, c // B
        if d == 0:
            in_maps.append({"a_xyz": xyz1[b], "b_xyz": xyz2[b], "ident": ident})
        else:
            in_maps.append({"a_xyz": xyz2[b], "b_xyz": xyz1[b], "ident": ident})

    res = run_bass_kernel_spmd(
        nc, in_maps, core_ids=list(range(2 * B)), trace=_trace
    )
    dist1 = np.stack([res.results[b]["dist"] for b in range(B)])
    dist2 = np.stack([res.results[B + b]["dist"] for b in range(B)])
    if _trace:
        return (dist1, dist2), res
    return dist1, dist2
